# revision 1
# baseline (speedup 1.0000x reference)
import os
import sys

sys.path.insert(0, "/opt/trn_rl_repo")
os.environ.setdefault("MYCRO_LOCAL_CACHE", "1")

import numpy as np

N_CORES = 8
P = 128  # partition / tile size

last_exec_time_ns = None


def _preprocess(rows, cols, vals, per_core, n_tiles, group_tiles, bank, nb):
    """Lay out edges into SPMD-uniform slot arrays for dma_gather + segsum.

    Block columns are ordered: group g -> source bank b -> tile t-in-group.
    Edge at slot (partition p, column j) has source cols (bank-relative
    int16 index at flat position j*128+p of idx16), weight val[p,j], and
    dest row dst[p,j] (0..127) within its tile.  Pad slots: idx=0, val=0.
    """
    E = rows.shape[0]
    n_groups = -(-n_tiles // group_tiles)
    core = rows // per_core
    loc = rows - core * per_core
    t_loc = loc // P
    d_loc = loc - t_loc * P
    g_loc = t_loc // group_tiles
    t_in_g = t_loc - g_loc * group_tiles
    # Chunked-allgather layout: bank b holds chunk b of every core's h.
    # Node (c, k, r) with k = chunk, r = row-in-chunk (rows_c =
    # per_core // nb) lives in bank k at row c*rows_c + r.
    rows_c = per_core // nb
    assert bank == N_CORES * rows_c
    cc = cols // per_core
    rem = cols - cc * per_core
    kk = rem // rows_c
    rr = rem - kk * rows_c
    h2 = kk * bank + cc * rows_c + rr
    b_src = kk

    key = ((core * n_groups + g_loc) * nb + b_src) * group_tiles + t_in_g
    nkeys = N_CORES * n_groups * nb * group_tiles
    order = np.argsort(key, kind="stable")
    counts = np.bincount(key, minlength=nkeys)
    cnt4 = counts.reshape(N_CORES, n_groups, nb, group_tiles)
    K = (cnt4.max(axis=0) + P - 1) // P  # [n_groups, nb, group_tiles]
    for g in range(n_groups):
        for ti in range(group_tiles):
            t = g * group_tiles + ti
            if t >= n_tiles:
                K[g, :, ti] = 0
            elif K[g, :, ti].sum() == 0:
                K[g, 0, ti] = 1

    col_of = np.zeros((n_groups, nb, group_tiles), dtype=np.int64)
    grp_base = np.zeros(n_groups + 1, dtype=np.int64)
    seg = np.zeros((n_groups, nb, 2), dtype=np.int64)  # (start, len)
    run = 0
    for g in range(n_groups):
        grp_base[g] = run
        for b in range(nb):
            seg[g, b, 0] = run
            for ti in range(group_tiles):
                col_of[g, b, ti] = run
                run += int(K[g, b, ti])
            seg[g, b, 1] = run - seg[g, b, 0]
    grp_base[n_groups] = run
    TOT = int(run)

    key_s = key[order]
    grp_start = np.concatenate([[0], np.cumsum(counts)])
    ranks = np.arange(E, dtype=np.int64) - grp_start[key_s]
    cs = core[order]
    gs = g_loc[order]
    bs = b_src[order]
    tis = t_in_g[order]
    p = ranks % P
    colj = col_of[gs, bs, tis] + ranks // P

    idx_flat = np.zeros((N_CORES, TOT * P), dtype=np.int16)
    idx_flat[cs, colj * P + p] = (h2[order] - bs * bank).astype(np.int16)
    val_a = np.zeros((N_CORES, P, TOT), dtype=np.float32)
    dst_a = np.zeros((N_CORES, P, TOT), dtype=np.float32)
    val_a[cs, p, colj] = vals[order]
    dst_a[cs, p, colj] = d_loc[order].astype(np.float32)

    # dma_gather index layout: flat i at partition i%16, col i//16,
    # replicated across the 8 groups of 16 partitions.
    idx16 = np.empty((N_CORES, P, TOT * 8), dtype=np.int16)
    for c in range(N_CORES):
        a = idx_flat[c].reshape(-1, 16).T
        idx16[c] = np.tile(a, (8, 1))

    tile_cols = []
    for t in range(n_tiles):
        g, ti = t // group_tiles, t % group_tiles
        cl = []
        for b in range(nb):
            c0 = int(col_of[g, b, ti])
            cl.extend(range(c0, c0 + int(K[g, b, ti])))
        tile_cols.append(cl)

    layout = dict(n_groups=n_groups, TOT=TOT, grp_base=grp_base, seg=seg,
                  tile_cols=tile_cols, K=K, idx_flat=idx_flat)
    return idx16, val_a, dst_a, layout


def _build_program(F1, F2, per_core, n_tiles, group_tiles, bank, nb, layout,
                   debug_dumps=False):
    import concourse.bass as bass
    import concourse.bacc as bacc
    import concourse.mybir as mybir
    import concourse.tile as tile

    fp32 = mybir.dt.float32
    i16 = mybir.dt.int16
    i32 = mybir.dt.int32
    NP_ = per_core * N_CORES
    AF = mybir.ActivationFunctionType
    OP = mybir.AluOpType

    TOT = layout["TOT"]
    grp_base = layout["grp_base"]
    seg = layout["seg"]
    tile_cols = layout["tile_cols"]
    n_groups = layout["n_groups"]

    nc = bacc.Bacc("TRN2", target_bir_lowering=False, debug=False,
                   num_devices=N_CORES)
    x_ext = nc.dram_tensor("x", [per_core, F1], fp32, kind="ExternalInput")
    w1_ext = nc.dram_tensor("w1", [F1, F1], fp32, kind="ExternalInput")
    b1_ext = nc.dram_tensor("b1", [1, F1], fp32, kind="ExternalInput")
    w2_ext = nc.dram_tensor("w2", [F1, F2], fp32, kind="ExternalInput")
    b2_ext = nc.dram_tensor("b2", [1, F2], fp32, kind="ExternalInput")
    idx16_ext = nc.dram_tensor("idx16", [P, TOT * 8], i16,
                               kind="ExternalInput")
    val_ext = nc.dram_tensor("val", [P, TOT], fp32, kind="ExternalInput")
    dst_ext = nc.dram_tensor("dst", [P, TOT], fp32, kind="ExternalInput")
    iotaf_ext = nc.dram_tensor("iotaf", [P, P], fp32, kind="ExternalInput")
    ident_ext = nc.dram_tensor("ident", [P, P], fp32, kind="ExternalInput")
    out_ext = nc.dram_tensor("out", [per_core, F2], fp32, kind="ExternalOutput")

    if debug_dumps:
        Lg0 = int(grp_base[1] - grp_base[0])
        hdbg_ext = nc.dram_tensor("hdbg", [per_core, F1], fp32,
                                  kind="ExternalOutput")
        Hfdbg_ext = nc.dram_tensor("Hfdbg", [NP_, F1], fp32,
                                   kind="ExternalOutput")
        mdbg_ext = nc.dram_tensor("mdbg", [P, Lg0 * F1], fp32,
                                  kind="ExternalOutput")
        y2dbg_ext = nc.dram_tensor("y2dbg", [per_core, F2], fp32,
                                   kind="ExternalOutput")

    with tile.TileContext(nc) as tc:
        with tc.tile_pool(name="static", bufs=1) as static, \
             tc.tile_pool(name="dram", bufs=1, space="DRAM") as dram:
            w1_sb = static.tile([F1, F1], fp32)
            nc.sync.dma_start(w1_sb[:], w1_ext[:])
            b1_sb = static.tile([1, F1], fp32)
            nc.sync.dma_start(b1_sb[:], b1_ext[:])
            w2_sb = static.tile([F1, F2], fp32)
            nc.sync.dma_start(w2_sb[:], w2_ext[:])
            b2_sb = static.tile([1, F2], fp32)
            nc.sync.dma_start(b2_sb[:], b2_ext[:])
            idx16_sb = static.tile([P, TOT * 8], i16)
            nc.sync.dma_start(idx16_sb[:], idx16_ext[:])
            val_sb = static.tile([P, TOT], fp32)
            nc.sync.dma_start(val_sb[:], val_ext[:])
            dst_sb = static.tile([P, TOT], fp32)
            nc.sync.dma_start(dst_sb[:], dst_ext[:])

            ones_sb = static.tile([1, P], fp32)
            nc.vector.memset(ones_sb[:], 1.0)
            # iota / identity come from host: avoids InstIota (library 0)
            # which deadlocks on HW when interleaved with dma_gather (lib 3).
            iota_f = static.tile([P, P], fp32)
            nc.sync.dma_start(iota_f[:], iotaf_ext[:])
            ident = static.tile([P, P], fp32)
            nc.sync.dma_start(ident[:], ident_ext[:])

            h_dram = dram.tile([per_core, F1], fp32)
            H_ch = [dram.tile([bank, F1], fp32, addr_space="Shared",
                              name=f"H_ch{k}") for k in range(nb)]
            y2_dram = dram.tile([per_core, F2], fp32)
            Y2_ch = [dram.tile([bank, F2], fp32, addr_space="Shared",
                               name=f"Y2_ch{k}") for k in range(nb)]

            # ---- Phase A: h = x @ W1 + b1  (own nodes) ----
            with tc.tile_pool(name="xa", bufs=2) as xa, \
                 tc.tile_pool(name="ha", bufs=2) as ha, \
                 tc.tile_pool(name="psA", bufs=2,
                              space=bass.MemorySpace.PSUM) as psA, \
                 tc.tile_pool(name="psB", bufs=2,
                              space=bass.MemorySpace.PSUM) as psB:
                for t in range(n_tiles):
                    x_sb = xa.tile([P, F1], fp32)
                    nc.sync.dma_start(x_sb[:], x_ext[t * P:(t + 1) * P, :])
                    xT_ps = psA.tile([F1, P], fp32)
                    nc.tensor.transpose(xT_ps[:], x_sb[:], ident[:])
                    xT_sb = ha.tile([F1, P], fp32)
                    nc.vector.tensor_copy(xT_sb[:], xT_ps[:])
                    h_ps = psB.tile([P, F1], fp32)
                    nc.tensor.matmul(h_ps[:], ones_sb[:], b1_sb[:],
                                     start=True, stop=False)
                    nc.tensor.matmul(h_ps[:], xT_sb[:], w1_sb[:],
                                     start=False, stop=True)
                    h_sb = ha.tile([P, F1], fp32)
                    nc.scalar.activation(h_sb[:], h_ps[:], AF.Copy)
                    nc.scalar.dma_start(h_dram[t * P:(t + 1) * P, :], h_sb[:])

            # ---- Phase B: AllGather h (chunked; big single collectives
            # crash the axon relay) ----
            rows_c = per_core // nb
            for k in range(nb):
                nc.gpsimd.collective_compute(
                    "AllGather", OP.bypass,
                    ins=[h_dram[k * rows_c:(k + 1) * rows_c, :]],
                    outs=[H_ch[k][:]],
                    replica_groups=[list(range(N_CORES))])
            if debug_dumps:
                nc.sync.dma_start(hdbg_ext[:], h_dram[:])
                nc.sync.dma_start(Hfdbg_ext[0:bank, :], H_ch[0][:])

            # ---- Phase C: z = relu(segsum L1); y2 = z @ W2 + b2 ----
            with tc.tile_pool(name="m1", bufs=2) as m1, \
                 tc.tile_pool(name="s1", bufs=4) as s1, \
                 tc.tile_pool(name="o1", bufs=2) as o1, \
                 tc.tile_pool(name="psZ", bufs=2,
                              space=bass.MemorySpace.PSUM) as psZ, \
                 tc.tile_pool(name="psT", bufs=2,
                              space=bass.MemorySpace.PSUM) as psT, \
                 tc.tile_pool(name="psY", bufs=2,
                              space=bass.MemorySpace.PSUM) as psY:
                for g in range(n_groups):
                    base = int(grp_base[g])
                    Lg = int(grp_base[g + 1]) - base
                    m_sb = m1.tile([P, Lg, F1], fp32)
                    for b in range(nb):
                        sA = int(seg[g, b, 0])
                        LA = int(seg[g, b, 1])
                        # chunk: >16 blocks (2048 idxs) overflows the
                        # 128-entry SWDGE descriptor ring -> device hang
                        for s0 in range(sA, sA + LA, 16):
                            L = min(16, sA + LA - s0)
                            nc.gpsimd.dma_gather(
                                m_sb[:, s0 - base:s0 - base + L, :],
                                H_ch[b][:],
                                idx16_sb[:, s0 * 8:(s0 + L) * 8],
                                L * P, L * P, F1)
                    if debug_dumps and g == 0:
                        nc.sync.dma_start(
                            mdbg_ext[:],
                            m_sb[:].rearrange("p a b -> p (a b)"))
                    for t in range(g * group_tiles,
                                   min((g + 1) * group_tiles, n_tiles)):
                        cl = tile_cols[t]
                        z_ps = psZ.tile([P, F1], fp32)
                        for i, j in enumerate(cl):
                            s_sb = s1.tile([P, P], fp32)
                            nc.vector.tensor_scalar(
                                out=s_sb[:], in0=iota_f[:],
                                scalar1=dst_sb[:, j:j + 1],
                                scalar2=val_sb[:, j:j + 1],
                                op0=OP.is_equal, op1=OP.mult)
                            nc.tensor.matmul(z_ps[:], s_sb[:],
                                             m_sb[:, j - base, :],
                                             start=(i == 0),
                                             stop=(i == len(cl) - 1))
                        z_sb = o1.tile([P, F1], fp32)
                        nc.scalar.activation(z_sb[:], z_ps[:], AF.Relu)
                        zT_ps = psT.tile([F1, P], fp32)
                        nc.tensor.transpose(zT_ps[:], z_sb[:], ident[:])
                        zT_sb = o1.tile([F1, P], fp32)
                        nc.scalar.activation(zT_sb[:], zT_ps[:], AF.Copy)
                        y2_ps = psY.tile([P, F2], fp32)
                        nc.tensor.matmul(y2_ps[:], ones_sb[:], b2_sb[:],
                                         start=True, stop=False)
                        nc.tensor.matmul(y2_ps[:], zT_sb[:], w2_sb[:],
                                         start=False, stop=True)
                        y2_sb = o1.tile([P, F2], fp32)
                        nc.scalar.activation(y2_sb[:], y2_ps[:], AF.Copy)
                        nc.scalar.dma_start(
                            y2_dram[t * P:(t + 1) * P, :], y2_sb[:])

            # ---- Phase D: AllGather y2 (chunked) ----
            for k in range(nb):
                nc.gpsimd.collective_compute(
                    "AllGather", OP.bypass,
                    ins=[y2_dram[k * rows_c:(k + 1) * rows_c, :]],
                    outs=[Y2_ch[k][:]],
                    replica_groups=[list(range(N_CORES))])
            if debug_dumps:
                nc.sync.dma_start(y2dbg_ext[:], y2_dram[:])

            # ---- Phase E: out = segsum L2 ----
            with tc.tile_pool(name="m2", bufs=2) as m2, \
                 tc.tile_pool(name="s2", bufs=4) as s2, \
                 tc.tile_pool(name="o2", bufs=2) as o2, \
                 tc.tile_pool(name="psO", bufs=2,
                              space=bass.MemorySpace.PSUM) as psO:
                for g in range(n_groups):
                    base = int(grp_base[g])
                    Lg = int(grp_base[g + 1]) - base
                    m_sb = m2.tile([P, Lg, F2], fp32)
                    for b in range(nb):
                        sA = int(seg[g, b, 0])
                        LA = int(seg[g, b, 1])
                        for s0 in range(sA, sA + LA, 16):
                            L = min(16, sA + LA - s0)
                            nc.gpsimd.dma_gather(
                                m_sb[:, s0 - base:s0 - base + L, :],
                                Y2_ch[b][:],
                                idx16_sb[:, s0 * 8:(s0 + L) * 8],
                                L * P, L * P, F2)
                    for t in range(g * group_tiles,
                                   min((g + 1) * group_tiles, n_tiles)):
                        cl = tile_cols[t]
                        o_ps = psO.tile([P, F2], fp32)
                        for i, j in enumerate(cl):
                            s_sb = s2.tile([P, P], fp32)
                            nc.vector.tensor_scalar(
                                out=s_sb[:], in0=iota_f[:],
                                scalar1=dst_sb[:, j:j + 1],
                                scalar2=val_sb[:, j:j + 1],
                                op0=OP.is_equal, op1=OP.mult)
                            nc.tensor.matmul(o_ps[:], s_sb[:],
                                             m_sb[:, j - base, :],
                                             start=(i == 0),
                                             stop=(i == len(cl) - 1))
                        o_sb = o2.tile([P, F2], fp32)
                        nc.scalar.activation(o_sb[:], o_ps[:], AF.Copy)
                        nc.scalar.dma_start(
                            out_ext[t * P:(t + 1) * P, :], o_sb[:])

    nc.compile()
    return nc


def _run(rows, cols, vals, x, W1, b1, W2, b2, group_tiles=3, bank_rows=None,
         trace=False, debug_dumps=False, full_results=False):
    from concourse.bass_utils import run_bass_kernel_spmd

    n_nodes, F1 = x.shape
    F2 = W2.shape[1]
    NP_ = -(-n_nodes // (N_CORES * P)) * (N_CORES * P)
    per_core = NP_ // N_CORES
    n_tiles = per_core // P

    if bank_rows is None:
        bank_rows = NP_ // (2 * N_CORES)
    assert NP_ % bank_rows == 0
    nb = NP_ // bank_rows
    assert bank_rows <= 32767
    assert per_core % nb == 0

    idx16, val_a, dst_a, layout = _preprocess(
        rows, cols, vals, per_core, n_tiles, group_tiles, bank_rows, nb)

    x_pad = np.zeros((NP_, F1), dtype=np.float32)
    x_pad[:n_nodes] = x
    b1r = np.ascontiguousarray(b1.reshape(1, F1).astype(np.float32))
    b2r = np.ascontiguousarray(b2.reshape(1, F2).astype(np.float32))

    nc = _build_program(F1, F2, per_core, n_tiles, group_tiles, bank_rows, nb,
                        layout, debug_dumps=debug_dumps)

    in_maps = []
    for c in range(N_CORES):
        in_maps.append({
            "x": np.ascontiguousarray(x_pad[c * per_core:(c + 1) * per_core]),
            "w1": np.ascontiguousarray(W1.astype(np.float32)),
            "b1": b1r,
            "w2": np.ascontiguousarray(W2.astype(np.float32)),
            "b2": b2r,
            "idx16": np.ascontiguousarray(idx16[c]),
            "val": np.ascontiguousarray(val_a[c]),
            "dst": np.ascontiguousarray(dst_a[c]),
            "iotaf": np.tile(np.arange(P, dtype=np.float32), (P, 1)),
            "ident": np.eye(P, dtype=np.float32),
        })

    import time as _time
    t0 = _time.perf_counter()
    res = run_bass_kernel_spmd(nc, in_maps, core_ids=list(range(N_CORES)),
                               trace=trace)
    wall_ns = int((_time.perf_counter() - t0) * 1e9)
    t_ns = res.exec_time_ns if res.exec_time_ns is not None else wall_ns
    out = np.concatenate([res.results[c]["out"] for c in range(N_CORES)],
                         axis=0)[:n_nodes]
    if full_results:
        return out, res, (idx16, val_a, dst_a, layout, per_core, n_tiles,
                          bank_rows, nb)
    return out, t_ns


def kernel(**inputs):
    global last_exec_time_ns
    trace = os.environ.get("KERNEL_TRACE", "0") == "1"
    out, t_ns = _run(inputs["rows"], inputs["cols"], inputs["vals"],
                     inputs["x"], inputs["W1"], inputs["b1"],
                     inputs["W2"], inputs["b2"], trace=trace)
    last_exec_time_ns = t_ns
    return out



# revision 3
# speedup vs baseline: 1.1043x; 1.1043x over previous
import os
import sys

sys.path.insert(0, "/opt/trn_rl_repo")
os.environ.setdefault("MYCRO_LOCAL_CACHE", "1")

import numpy as np

N_CORES = 8
P = 128  # partition / tile size

last_exec_time_ns = None


def _preprocess(rows, cols, vals, per_core, n_tiles, group_tiles, bank, nb):
    """Lay out edges into SPMD-uniform slot arrays for dma_gather + segsum.

    Block columns are ordered: group g -> source bank b -> tile t-in-group.
    Edge at slot (partition p, column j) has source cols (bank-relative
    int16 index at flat position j*128+p of idx16), weight val[p,j], and
    dest row dst[p,j] (0..127) within its tile.  Pad slots: idx=0, val=0.
    """
    E = rows.shape[0]
    n_groups = -(-n_tiles // group_tiles)
    core = rows // per_core
    loc = rows - core * per_core
    t_loc = loc // P
    d_loc = loc - t_loc * P
    g_loc = t_loc // group_tiles
    t_in_g = t_loc - g_loc * group_tiles
    # Chunked-allgather layout: bank b holds chunk b of every core's h.
    # Node (c, k, r) with k = chunk, r = row-in-chunk (rows_c =
    # per_core // nb) lives in bank k at row c*rows_c + r.
    rows_c = per_core // nb
    assert bank == N_CORES * rows_c
    cc = cols // per_core
    rem = cols - cc * per_core
    kk = rem // rows_c
    rr = rem - kk * rows_c
    h2 = kk * bank + cc * rows_c + rr
    b_src = kk

    key = ((core * n_groups + g_loc) * nb + b_src) * group_tiles + t_in_g
    nkeys = N_CORES * n_groups * nb * group_tiles
    order = np.argsort(key, kind="stable")
    counts = np.bincount(key, minlength=nkeys)
    cnt4 = counts.reshape(N_CORES, n_groups, nb, group_tiles)
    K = (cnt4.max(axis=0) + P - 1) // P  # [n_groups, nb, group_tiles]
    for g in range(n_groups):
        for ti in range(group_tiles):
            t = g * group_tiles + ti
            if t >= n_tiles:
                K[g, :, ti] = 0
            elif K[g, :, ti].sum() == 0:
                K[g, 0, ti] = 1

    col_of = np.zeros((n_groups, nb, group_tiles), dtype=np.int64)
    grp_base = np.zeros(n_groups + 1, dtype=np.int64)
    seg = np.zeros((n_groups, nb, 2), dtype=np.int64)  # (start, len)
    run = 0
    for g in range(n_groups):
        grp_base[g] = run
        for b in range(nb):
            seg[g, b, 0] = run
            for ti in range(group_tiles):
                col_of[g, b, ti] = run
                run += int(K[g, b, ti])
            seg[g, b, 1] = run - seg[g, b, 0]
    grp_base[n_groups] = run
    TOT = int(run)

    key_s = key[order]
    grp_start = np.concatenate([[0], np.cumsum(counts)])
    ranks = np.arange(E, dtype=np.int64) - grp_start[key_s]
    cs = core[order]
    gs = g_loc[order]
    bs = b_src[order]
    tis = t_in_g[order]
    p = ranks % P
    colj = col_of[gs, bs, tis] + ranks // P

    idx_flat = np.zeros((N_CORES, TOT * P), dtype=np.int16)
    idx_flat[cs, colj * P + p] = (h2[order] - bs * bank).astype(np.int16)
    val_a = np.zeros((N_CORES, P, TOT), dtype=np.float32)
    dst_a = np.zeros((N_CORES, P, TOT), dtype=np.float32)
    val_a[cs, p, colj] = vals[order]
    dst_a[cs, p, colj] = d_loc[order].astype(np.float32)

    # dma_gather index layout: flat i at partition i%16, col i//16,
    # replicated across the 8 groups of 16 partitions.
    idx16 = np.empty((N_CORES, P, TOT * 8), dtype=np.int16)
    for c in range(N_CORES):
        a = idx_flat[c].reshape(-1, 16).T
        idx16[c] = np.tile(a, (8, 1))

    tile_cols = []
    for t in range(n_tiles):
        g, ti = t // group_tiles, t % group_tiles
        cl = []
        for b in range(nb):
            c0 = int(col_of[g, b, ti])
            cl.extend(range(c0, c0 + int(K[g, b, ti])))
        tile_cols.append(cl)

    layout = dict(n_groups=n_groups, TOT=TOT, grp_base=grp_base, seg=seg,
                  tile_cols=tile_cols, K=K, idx_flat=idx_flat)
    return idx16, val_a, dst_a, layout


def _build_program(F1, F2, per_core, n_tiles, group_tiles, bank, nb, layout,
                   debug_dumps=False):
    import concourse.bass as bass
    import concourse.bacc as bacc
    import concourse.mybir as mybir
    import concourse.tile as tile

    fp32 = mybir.dt.float32
    i16 = mybir.dt.int16
    i32 = mybir.dt.int32
    NP_ = per_core * N_CORES
    AF = mybir.ActivationFunctionType
    OP = mybir.AluOpType

    TOT = layout["TOT"]
    grp_base = layout["grp_base"]
    seg = layout["seg"]
    tile_cols = layout["tile_cols"]
    n_groups = layout["n_groups"]

    nc = bacc.Bacc("TRN2", target_bir_lowering=False, debug=False,
                   num_devices=N_CORES)
    x_ext = nc.dram_tensor("x", [per_core, F1], fp32, kind="ExternalInput")
    w1_ext = nc.dram_tensor("w1", [F1, F1], fp32, kind="ExternalInput")
    b1_ext = nc.dram_tensor("b1", [1, F1], fp32, kind="ExternalInput")
    w2_ext = nc.dram_tensor("w2", [F1, F2], fp32, kind="ExternalInput")
    b2_ext = nc.dram_tensor("b2", [1, F2], fp32, kind="ExternalInput")
    idx16_ext = nc.dram_tensor("idx16", [P, TOT * 8], i16,
                               kind="ExternalInput")
    val_ext = nc.dram_tensor("val", [P, TOT], fp32, kind="ExternalInput")
    dst_ext = nc.dram_tensor("dst", [P, TOT], fp32, kind="ExternalInput")
    iotaf_ext = nc.dram_tensor("iotaf", [P, P], fp32, kind="ExternalInput")
    ident_ext = nc.dram_tensor("ident", [P, P], fp32, kind="ExternalInput")
    out_ext = nc.dram_tensor("out", [per_core, F2], fp32, kind="ExternalOutput")

    if debug_dumps:
        Lg0 = int(grp_base[1] - grp_base[0])
        hdbg_ext = nc.dram_tensor("hdbg", [per_core, F1], fp32,
                                  kind="ExternalOutput")
        Hfdbg_ext = nc.dram_tensor("Hfdbg", [NP_, F1], fp32,
                                   kind="ExternalOutput")
        mdbg_ext = nc.dram_tensor("mdbg", [P, Lg0 * F1], fp32,
                                  kind="ExternalOutput")
        y2dbg_ext = nc.dram_tensor("y2dbg", [per_core, F2], fp32,
                                   kind="ExternalOutput")

    with tile.TileContext(nc) as tc:
        with tc.tile_pool(name="static", bufs=1) as static, \
             tc.tile_pool(name="dram", bufs=1, space="DRAM") as dram:
            w1_sb = static.tile([F1, F1], fp32)
            nc.sync.dma_start(w1_sb[:], w1_ext[:])
            b1_sb = static.tile([1, F1], fp32)
            nc.sync.dma_start(b1_sb[:], b1_ext[:])
            w2_sb = static.tile([F1, F2], fp32)
            nc.sync.dma_start(w2_sb[:], w2_ext[:])
            b2_sb = static.tile([1, F2], fp32)
            nc.sync.dma_start(b2_sb[:], b2_ext[:])
            idx16_sb = static.tile([P, TOT * 8], i16)
            nc.sync.dma_start(idx16_sb[:], idx16_ext[:])
            val_sb = static.tile([P, TOT], fp32)
            nc.sync.dma_start(val_sb[:], val_ext[:])
            dst_sb = static.tile([P, TOT], fp32)
            nc.sync.dma_start(dst_sb[:], dst_ext[:])

            ones_sb = static.tile([1, P], fp32)
            nc.vector.memset(ones_sb[:], 1.0)
            # iota / identity come from host: avoids InstIota (library 0)
            # which deadlocks on HW when interleaved with dma_gather (lib 3).
            iota_f = static.tile([P, P], fp32)
            nc.sync.dma_start(iota_f[:], iotaf_ext[:])
            ident = static.tile([P, P], fp32)
            nc.sync.dma_start(ident[:], ident_ext[:])

            h_dram = dram.tile([per_core, F1], fp32)
            H_ch = [dram.tile([bank, F1], fp32, addr_space="Shared",
                              name=f"H_ch{k}") for k in range(nb)]
            y2_dram = dram.tile([per_core, F2], fp32)
            Y2_ch = [dram.tile([bank, F2], fp32, addr_space="Shared",
                               name=f"Y2_ch{k}") for k in range(nb)]

            # ---- Phase A: h = x @ W1 + b1  (own nodes) ----
            with tc.tile_pool(name="xa", bufs=2) as xa, \
                 tc.tile_pool(name="ha", bufs=2) as ha, \
                 tc.tile_pool(name="psA", bufs=2,
                              space=bass.MemorySpace.PSUM) as psA, \
                 tc.tile_pool(name="psB", bufs=2,
                              space=bass.MemorySpace.PSUM) as psB:
                for t in range(n_tiles):
                    x_sb = xa.tile([P, F1], fp32)
                    nc.sync.dma_start(x_sb[:], x_ext[t * P:(t + 1) * P, :])
                    xT_ps = psA.tile([F1, P], fp32)
                    nc.tensor.transpose(xT_ps[:], x_sb[:], ident[:])
                    xT_sb = ha.tile([F1, P], fp32)
                    nc.vector.tensor_copy(xT_sb[:], xT_ps[:])
                    h_ps = psB.tile([P, F1], fp32)
                    nc.tensor.matmul(h_ps[:], ones_sb[:], b1_sb[:],
                                     start=True, stop=False)
                    nc.tensor.matmul(h_ps[:], xT_sb[:], w1_sb[:],
                                     start=False, stop=True)
                    h_sb = ha.tile([P, F1], fp32)
                    nc.scalar.activation(h_sb[:], h_ps[:], AF.Copy)
                    nc.scalar.dma_start(h_dram[t * P:(t + 1) * P, :], h_sb[:])

            # ---- Phase B: AllGather h (chunked; big single collectives
            # crash the axon relay) ----
            rows_c = per_core // nb
            for k in range(nb):
                nc.gpsimd.collective_compute(
                    "AllGather", OP.bypass,
                    ins=[h_dram[k * rows_c:(k + 1) * rows_c, :]],
                    outs=[H_ch[k][:]],
                    replica_groups=[list(range(N_CORES))])
            if debug_dumps:
                nc.sync.dma_start(hdbg_ext[:], h_dram[:])
                nc.sync.dma_start(Hfdbg_ext[0:bank, :], H_ch[0][:])

            # ---- Phase C: z = relu(segsum L1); y2 = z @ W2 + b2 ----
            with tc.tile_pool(name="m1", bufs=2) as m1, \
                 tc.tile_pool(name="s1", bufs=4) as s1, \
                 tc.tile_pool(name="o1", bufs=2) as o1, \
                 tc.tile_pool(name="psZ", bufs=2,
                              space=bass.MemorySpace.PSUM) as psZ, \
                 tc.tile_pool(name="psT", bufs=2,
                              space=bass.MemorySpace.PSUM) as psT, \
                 tc.tile_pool(name="psY", bufs=2,
                              space=bass.MemorySpace.PSUM) as psY:
                for g in range(n_groups):
                    base = int(grp_base[g])
                    Lg = int(grp_base[g + 1]) - base
                    m_sb = m1.tile([P, Lg, F1], fp32)
                    for b in range(nb):
                        sA = int(seg[g, b, 0])
                        LA = int(seg[g, b, 1])
                        # chunk: >16 blocks (2048 idxs) overflows the
                        # 128-entry SWDGE descriptor ring -> device hang
                        for s0 in range(sA, sA + LA, 16):
                            L = min(16, sA + LA - s0)
                            nc.gpsimd.dma_gather(
                                m_sb[:, s0 - base:s0 - base + L, :],
                                H_ch[b][:],
                                idx16_sb[:, s0 * 8:(s0 + L) * 8],
                                L * P, L * P, F1)
                    if debug_dumps and g == 0:
                        nc.sync.dma_start(
                            mdbg_ext[:],
                            m_sb[:].rearrange("p a b -> p (a b)"))
                    for t in range(g * group_tiles,
                                   min((g + 1) * group_tiles, n_tiles)):
                        cl = tile_cols[t]
                        z_ps = psZ.tile([P, F1], fp32)
                        for i, j in enumerate(cl):
                            s_sb = s1.tile([P, P], fp32)
                            nc.vector.tensor_scalar(
                                out=s_sb[:], in0=iota_f[:],
                                scalar1=dst_sb[:, j:j + 1],
                                scalar2=val_sb[:, j:j + 1],
                                op0=OP.is_equal, op1=OP.mult)
                            nc.tensor.matmul(z_ps[:], s_sb[:],
                                             m_sb[:, j - base, :],
                                             start=(i == 0),
                                             stop=(i == len(cl) - 1))
                        z_sb = o1.tile([P, F1], fp32)
                        nc.scalar.activation(z_sb[:], z_ps[:], AF.Relu)
                        zT_ps = psT.tile([F1, P], fp32)
                        nc.tensor.transpose(zT_ps[:], z_sb[:], ident[:])
                        zT_sb = o1.tile([F1, P], fp32)
                        nc.scalar.activation(zT_sb[:], zT_ps[:], AF.Copy)
                        y2_ps = psY.tile([P, F2], fp32)
                        nc.tensor.matmul(y2_ps[:], ones_sb[:], b2_sb[:],
                                         start=True, stop=False)
                        nc.tensor.matmul(y2_ps[:], zT_sb[:], w2_sb[:],
                                         start=False, stop=True)
                        y2_sb = o1.tile([P, F2], fp32)
                        nc.scalar.activation(y2_sb[:], y2_ps[:], AF.Copy)
                        nc.scalar.dma_start(
                            y2_dram[t * P:(t + 1) * P, :], y2_sb[:])

            # ---- Phase D: AllGather y2 (chunked) ----
            for k in range(nb):
                nc.gpsimd.collective_compute(
                    "AllGather", OP.bypass,
                    ins=[y2_dram[k * rows_c:(k + 1) * rows_c, :]],
                    outs=[Y2_ch[k][:]],
                    replica_groups=[list(range(N_CORES))])
            if debug_dumps:
                nc.sync.dma_start(y2dbg_ext[:], y2_dram[:])

            # ---- Phase E: out = segsum L2 ----
            with tc.tile_pool(name="m2", bufs=2) as m2, \
                 tc.tile_pool(name="s2", bufs=4) as s2, \
                 tc.tile_pool(name="o2", bufs=2) as o2, \
                 tc.tile_pool(name="psO", bufs=2,
                              space=bass.MemorySpace.PSUM) as psO:
                for g in range(n_groups):
                    base = int(grp_base[g])
                    Lg = int(grp_base[g + 1]) - base
                    m_sb = m2.tile([P, Lg, F2], fp32)
                    for b in range(nb):
                        sA = int(seg[g, b, 0])
                        LA = int(seg[g, b, 1])
                        for s0 in range(sA, sA + LA, 16):
                            L = min(16, sA + LA - s0)
                            nc.gpsimd.dma_gather(
                                m_sb[:, s0 - base:s0 - base + L, :],
                                Y2_ch[b][:],
                                idx16_sb[:, s0 * 8:(s0 + L) * 8],
                                L * P, L * P, F2)
                    for t in range(g * group_tiles,
                                   min((g + 1) * group_tiles, n_tiles)):
                        cl = tile_cols[t]
                        o_ps = psO.tile([P, F2], fp32)
                        for i, j in enumerate(cl):
                            s_sb = s2.tile([P, P], fp32)
                            nc.vector.tensor_scalar(
                                out=s_sb[:], in0=iota_f[:],
                                scalar1=dst_sb[:, j:j + 1],
                                scalar2=val_sb[:, j:j + 1],
                                op0=OP.is_equal, op1=OP.mult)
                            nc.tensor.matmul(o_ps[:], s_sb[:],
                                             m_sb[:, j - base, :],
                                             start=(i == 0),
                                             stop=(i == len(cl) - 1))
                        o_sb = o2.tile([P, F2], fp32)
                        nc.scalar.activation(o_sb[:], o_ps[:], AF.Copy)
                        nc.scalar.dma_start(
                            out_ext[t * P:(t + 1) * P, :], o_sb[:])

    nc.compile()
    return nc


def _run(rows, cols, vals, x, W1, b1, W2, b2, group_tiles=3, bank_rows=None,
         trace=False, debug_dumps=False, full_results=False):
    from concourse.bass_utils import run_bass_kernel_spmd

    n_nodes, F1 = x.shape
    F2 = W2.shape[1]
    NP_ = -(-n_nodes // (N_CORES * P)) * (N_CORES * P)
    per_core = NP_ // N_CORES
    n_tiles = per_core // P

    if bank_rows is None:
        bank_rows = NP_ // (2 * N_CORES)
    assert NP_ % bank_rows == 0
    nb = NP_ // bank_rows
    assert bank_rows <= 32767
    assert per_core % nb == 0

    idx16, val_a, dst_a, layout = _preprocess(
        rows, cols, vals, per_core, n_tiles, group_tiles, bank_rows, nb)

    x_pad = np.zeros((NP_, F1), dtype=np.float32)
    x_pad[:n_nodes] = x
    b1r = np.ascontiguousarray(b1.reshape(1, F1).astype(np.float32))
    b2r = np.ascontiguousarray(b2.reshape(1, F2).astype(np.float32))

    nc = _build_program(F1, F2, per_core, n_tiles, group_tiles, bank_rows, nb,
                        layout, debug_dumps=debug_dumps)

    in_maps = []
    for c in range(N_CORES):
        in_maps.append({
            "x": np.ascontiguousarray(x_pad[c * per_core:(c + 1) * per_core]),
            "w1": np.ascontiguousarray(W1.astype(np.float32)),
            "b1": b1r,
            "w2": np.ascontiguousarray(W2.astype(np.float32)),
            "b2": b2r,
            "idx16": np.ascontiguousarray(idx16[c]),
            "val": np.ascontiguousarray(val_a[c]),
            "dst": np.ascontiguousarray(dst_a[c]),
            "iotaf": np.tile(np.arange(P, dtype=np.float32), (P, 1)),
            "ident": np.eye(P, dtype=np.float32),
        })

    import time as _time
    t0 = _time.perf_counter()
    res = run_bass_kernel_spmd(nc, in_maps, core_ids=list(range(N_CORES)),
                               trace=trace)
    wall_ns = int((_time.perf_counter() - t0) * 1e9)
    t_ns = res.exec_time_ns if res.exec_time_ns is not None else wall_ns
    out = np.concatenate([res.results[c]["out"] for c in range(N_CORES)],
                         axis=0)[:n_nodes]
    if full_results:
        return out, res, (idx16, val_a, dst_a, layout, per_core, n_tiles,
                          bank_rows, nb)
    return out, t_ns


def kernel(**inputs):
    global last_exec_time_ns
    trace = os.environ.get("KERNEL_TRACE", "0") == "1"
    out, t_ns = _run(inputs["rows"], inputs["cols"], inputs["vals"],
                     inputs["x"], inputs["W1"], inputs["b1"],
                     inputs["W2"], inputs["b2"], trace=trace)
    last_exec_time_ns = t_ns
    return out

